# revision 42
# baseline (speedup 1.0000x reference)
"""Trainium2 Bass kernel for nn_CentroidUOMNet (GNN message passing).

Data-parallel over target ids: 8 cores x 512 targets. Per core, layer-1
processes 16384 (target, member) pairs in 128-row tiles: indirect-DMA row
gather of sampled neighbor features, PE-transpose + Wu0 GEMM + relu +
k-reduce for the unorder-mapping, a masked-replication block-diagonal
matmul to apply the per-pair 8x8 mixing (emitting ue pre-transposed into
conv layout), then the length-3 conv as 48-row partition-window matmuls
accumulated over the 8 ue rows in PSUM. Layer 2 repeats the structure on
512 targets; the head does log_softmax via transpose + free-dim reduce.
"""
import os
import sys
import numpy as np

sys.path.insert(0, "/opt/trn_rl_repo")

N, DEG, K, D0, D1, D2, NCLS, NNODES = 4096, 32, 8, 128, 128, 128, 40, 100000
NCORES = 8
NC_N = N // NCORES       # 512 targets/core
B_C = NC_N * DEG         # 16384 pairs/core
SUB1 = B_C // 16         # 1024 sub-tiles layer1
SUB2 = NC_N // 16        # 32 sub-tiles layer2
CHUNK = 512              # pairs per conv chunk
SIG = np.array([8 * (g % 16) + g // 16 for g in range(128)])

_CACHE = {}


def _consts(inputs):
    import ml_dtypes
    bf16 = ml_dtypes.bfloat16
    f32 = np.float32
    Wu0, Wu1 = np.asarray(inputs["Wu0"], f32), np.asarray(inputs["Wu1"], f32)
    Wc0, Wc1 = np.asarray(inputs["Wc0"], f32), np.asarray(inputs["Wc1"], f32)
    wu0p = np.ascontiguousarray(Wu0[SIG], f32)            # [128,64], q = i*8+j
    wu1p = np.ascontiguousarray(Wu1[SIG], f32)
    wci = np.zeros((48, 8 * 128), f32)
    wci2 = np.zeros((48, 8 * 128), f32)
    for i in range(8):
        for t in range(3):
            for r in range(16):
                wci[t * 16 + r, i * 128:(i + 1) * 128] = Wc0[SIG, i * 16 + r, t] / 8.0
                wci2[t * 16 + r, i * 128:(i + 1) * 128] = Wc1[:, i * 16 + r, t] / 8.0
    # Zero-padded full-128-row conv weights, one per output position l:
    # row l*16+tr holds the window weight; matmul contracts all 128 u rows.
    wcp = np.zeros((6, 128, 1024), f32)
    wcp2 = np.zeros((6, 128, 1024), f32)
    for l in range(6):
        wcp[l, l * 16:l * 16 + 48, :] = wci
        wcp2[l, l * 16:l * 16 + 48, :] = wci2
    repl64 = np.zeros((64, 128), f32)
    for q in range(64):
        for b in range(16):
            repl64[q, b * 8 + (q % 8)] = 1.0
    maski = np.zeros((64, 512), f32)
    for q in range(64):
        for t in range(4):
            base = t * 128 + (q // 8) * 16
            maski[q, base:base + 16] = 1.0
    mask = np.zeros((128, 512), f32)
    for b in range(16):
        for j in range(8):
            for i in range(8):
                for t in range(4):
                    mask[b * 8 + j, t * 128 + i * 16 + b] = 1.0
    d = dict(
        fp=np.ascontiguousarray(np.asarray(inputs["feats"], f32)[:, SIG]).astype(bf16),
        wu0p=wu0p.astype(bf16), wu1p=wu1p.astype(bf16),
        repl64=repl64.astype(bf16), maski=maski.astype(bf16),
        mask=mask.astype(bf16),
        bu0p=np.asarray(inputs["bu0"], f32).reshape(64, 1),
        bu1p=np.asarray(inputs["bu1"], f32).reshape(64, 1),
        bc0p=(np.asarray(inputs["bc0"], f32)[SIG] / 6.0).reshape(128, 1),
        wf=np.asarray(inputs["Wf"], f32),
        bf=np.asarray(inputs["bf"], f32).reshape(40, 1),
        ident=np.eye(128, dtype=f32),
        identb=np.eye(128, dtype=f32).astype(bf16),
    )
    for l in range(6):
        d[f"wcp{l}"] = np.ascontiguousarray(wcp[l])
        d[f"wcq{l}"] = np.ascontiguousarray(wcp2[l])
    return d


def _indices(inputs):
    edge_dict = np.asarray(inputs["edge_dict"])
    ids = np.asarray(inputs["ids"])
    samp1 = np.asarray(inputs["samp1"])
    samp2 = np.asarray(inputs["samp2"])
    nb = edge_dict[ids]
    sel = np.take_along_axis(edge_dict[nb], samp1, axis=2)
    sel_flat = sel.reshape(N * DEG, K).astype(np.int32)
    per_core = []
    for c in range(NCORES):
        sl = sel_flat[c * B_C:(c + 1) * B_C]
        selT = np.ascontiguousarray(
            sl.reshape(SUB1, 16, 8).transpose(1, 2, 0).reshape(128, SUB1), np.int32)
        s2 = samp2[c * NC_N:(c + 1) * NC_N].astype(np.int32)
        sel2 = np.arange(NC_N, dtype=np.int32)[:, None] * DEG + s2
        sel2T = np.ascontiguousarray(
            sel2.reshape(SUB2, 16, 8).transpose(1, 2, 0).reshape(128, SUB2), np.int32)
        per_core.append((selT, sel2T))
    return per_core


def _emit_layer(nc, tc, pools, cst, src_dram, selT_sb, n_sub, wu_sb, bu_sb,
                wc_sb, layer1, bc_sb, ne_out, mybir, bass):
    """Emit one recursion layer. layer1: relu+mean -> ne_out (DRAM, transposed
    write). else: logsoftmax-mean -> returns embs sbuf tile."""
    dt = mybir.dt
    csb, psum, work = pools
    n_chunk = n_sub * 16 // CHUNK
    embs = None
    for ch in range(n_chunk):
        u_sb = work.tile([128, 8 * CHUNK], dt.float32r, tag="u_sb", name="u_sb", bufs=2)
        for grp in range(CHUNK // 128):        # 128-pair groups: 8 sub-tiles
            g = ch * (CHUNK // 128) + grp
            se8 = work.tile([128, 1024], dt.bfloat16, tag="se8", name="se8", bufs=8)
            nc.gpsimd.indirect_dma_start(
                out=se8[:], out_offset=None, in_=src_dram[:],
                in_offset=bass.IndirectOffsetOnAxis(
                    ap=selT_sb[:, g * 8:(g + 1) * 8], axis=0))
            seT_ps = psum.tile([128, 512], dt.bfloat16, tag="seT_ps", bufs=2)
            seT8 = work.tile([128, 1024], dt.bfloat16, tag="seT8")
            for half in range(2):
                for t4 in range(4):
                    t = half * 4 + t4
                    nc.tensor.transpose(
                        out=seT_ps[:, t4 * 128:(t4 + 1) * 128],
                        in_=se8[:, t * 128:(t + 1) * 128],
                        identity=cst["identb"][:])
                nc.scalar.copy(
                    out=seT8[:, half * 512:(half + 1) * 512], in_=seT_ps[:])
                if half == 0:
                    seT_ps = psum.tile([128, 512], dt.bfloat16, tag="seT_ps", bufs=2)
            for half in range(2):
                m_ps = psum.tile([64, 512], dt.float32, tag="m_ps", name="m_ps")
                nc.tensor.matmul(
                    out=m_ps[:], lhsT=wu_sb[:],
                    rhs=seT8[:, half * 512:(half + 1) * 512],
                    start=True, stop=True)
                r_sb = work.tile([64, 512], dt.bfloat16, tag="r_sb", name="r_sb")
                nc.scalar.activation(out=r_sb[:], in_=m_ps[:],
                                     func=mybir.ActivationFunctionType.Relu,
                                     bias=bu_sb[:], scale=1.0)
                mall = work.tile([64, 64], dt.bfloat16, tag="mall", name="mall")
                nc.vector.tensor_reduce(
                    out=mall[:], in_=r_sb[:].rearrange("p (c k) -> p c k", k=8),
                    axis=mybir.AxisListType.X, op=mybir.AluOpType.add)
                m2 = work.tile([64, 512], dt.bfloat16, tag="m2", name="m2")
                nc.vector.tensor_mul(
                    out=m2[:].rearrange("p (t i c) -> p t i c", t=4, i=8),
                    in0=mall[:].rearrange("p (t x c) -> p t x c", t=4, x=1
                                          ).to_broadcast([64, 4, 8, 16]),
                    in1=cst["maski"][:].rearrange("p (t i c) -> p t i c", t=4, i=8))
                d_ps = psum.tile([128, 512], dt.float32, tag="d_ps", name="d_ps")
                nc.tensor.matmul(out=d_ps[:], lhsT=cst["repl64"][:],
                                 rhs=m2[:], start=True, stop=True)
                bd_sb = work.tile([128, 512], dt.bfloat16, tag="bd_sb", name="bd_sb")
                nc.vector.tensor_mul(out=bd_sb[:], in0=d_ps[:], in1=cst["mask"][:])
                u_ps = psum.tile([128, 512], dt.float32, tag="u_ps", name="u_ps", bufs=2)
                for t4 in range(4):
                    t = half * 4 + t4
                    nc.tensor.matmul(out=u_ps[:, t4 * 128:(t4 + 1) * 128],
                                     lhsT=se8[:, t * 128:(t + 1) * 128],
                                     rhs=bd_sb[:, t4 * 128:(t4 + 1) * 128],
                                     start=True, stop=True)
                dst = u_sb[:].rearrange("p (i c) -> p i c", c=CHUNK)[
                    :, :, grp * 128 + half * 64: grp * 128 + (half + 1) * 64
                    ].rearrange("p i (t c) -> p i t c", t=4)
                eng = nc.scalar.copy if half == 0 else nc.vector.tensor_copy
                eng(out=dst,
                    in_=u_ps[:].rearrange("p (t i c) -> p i t c", t=4, i=8))
        # conv over this chunk: zero-padded 128-row weights, no shifted copies
        if layer1:
            ne_acc = work.tile([128, CHUNK], dt.bfloat16, tag="ne_acc", name="ne_acc", bufs=2)
            tmp = None
            for l in range(6):
                c_ps = psum.tile([128, CHUNK], dt.float32, tag="c_ps", name="c_ps")
                for i in range(8):
                    nc.tensor.matmul(
                        out=c_ps[:], lhsT=wc_sb[l][:, i * 128:(i + 1) * 128],
                        rhs=u_sb[:, i * CHUNK:(i + 1) * CHUNK],
                        start=(i == 0), stop=(i == 7))
                dst = ne_acc if l == 0 else (tmp := work.tile([128, CHUNK], dt.bfloat16, tag="cv_tmp", name="cv_tmp", bufs=2))
                nc.scalar.activation(out=dst[:], in_=c_ps[:],
                                     func=mybir.ActivationFunctionType.Relu,
                                     bias=bc_sb[:], scale=1.0 / 6.0)
                if l > 0:
                    nc.vector.tensor_add(out=ne_acc[:], in0=ne_acc[:], in1=tmp[:])
            ntp = work.tile([128, CHUNK], dt.bfloat16, tag="ntp", name="ntp", bufs=2)
            for q in range(CHUNK // 128):
                nt_ps = psum.tile([128, 128], dt.bfloat16, tag="nt_ps", name="nt_ps")
                nc.tensor.transpose(out=nt_ps[:],
                                    in_=ne_acc[:, q * 128:(q + 1) * 128],
                                    identity=cst["identb"][:])
                nc.scalar.copy(out=ntp[:, q * 128:(q + 1) * 128], in_=nt_ps[:])
            nc.sync.dma_start(
                out=ne_out[ch * CHUNK:(ch + 1) * CHUNK, :].rearrange(
                    "(q p) c -> p q c", q=4),
                in_=ntp[:].rearrange("p (q c) -> p q c", q=4))
        else:
            c_sb = []
            for l in range(6):
                c_ps = psum.tile([128, CHUNK], dt.float32, tag="c_ps", name="c_ps")
                for i in range(8):
                    nc.tensor.matmul(
                        out=c_ps[:], lhsT=wc_sb[l][:, i * 128:(i + 1) * 128],
                        rhs=u_sb[:, i * CHUNK:(i + 1) * CHUNK],
                        start=(i == 0), stop=(i == 7))
                t = work.tile([128, CHUNK], dt.float32, tag=f"c2_{l}", name=f"c2_{l}", bufs=1)
                nc.scalar.copy(out=t[:], in_=c_ps[:])
                c_sb.append(t)
            mx = work.tile([128, CHUNK], dt.float32, tag="mx", name="mx", bufs=1)
            nc.vector.tensor_max(out=mx[:], in0=c_sb[0][:], in1=c_sb[1][:])
            for l in range(2, 6):
                nc.vector.tensor_max(out=mx[:], in0=mx[:], in1=c_sb[l][:])
            esum = work.tile([128, CHUNK], dt.float32, tag="esum", name="esum", bufs=1)
            csum = work.tile([128, CHUNK], dt.float32, tag="csum", name="csum", bufs=1)
            for l in range(6):
                d = work.tile([128, CHUNK], dt.float32, tag="lsm_d", name="lsm_d", bufs=1)
                nc.vector.tensor_sub(out=d[:], in0=c_sb[l][:], in1=mx[:])
                e = work.tile([128, CHUNK], dt.float32, tag="lsm_e", name="lsm_e", bufs=1)
                nc.scalar.activation(out=e[:], in_=d[:],
                                     func=mybir.ActivationFunctionType.Exp)
                if l == 0:
                    nc.vector.tensor_copy(out=esum[:], in_=e[:])
                    nc.vector.tensor_copy(out=csum[:], in_=c_sb[0][:])
                else:
                    nc.vector.tensor_add(out=esum[:], in0=esum[:], in1=e[:])
                    nc.vector.tensor_add(out=csum[:], in0=csum[:], in1=c_sb[l][:])
            lg = work.tile([128, CHUNK], dt.float32, tag="lg", name="lg")
            nc.scalar.activation(out=lg[:], in_=esum[:],
                                 func=mybir.ActivationFunctionType.Ln)
            embs = work.tile([128, CHUNK], dt.float32r, tag="embs", name="embs", bufs=1)
            nc.vector.tensor_scalar_mul(out=embs[:], in0=csum[:], scalar1=1.0 / 6.0)
            nc.vector.tensor_sub(out=embs[:], in0=embs[:], in1=mx[:])
            nc.vector.tensor_sub(out=embs[:], in0=embs[:], in1=lg[:])
    return embs


def _build():
    import concourse.bass as bass
    import concourse.bacc as bacc
    import concourse.mybir as mybir
    import concourse.tile as tile

    dt = mybir.dt
    nc = bacc.Bacc("TRN2", target_bir_lowering=False, debug=False)
    fp_d = nc.dram_tensor("fp", [NNODES, 128], dt.bfloat16, kind="ExternalInput")
    selT_d = nc.dram_tensor("selT", [128, SUB1], dt.int32, kind="ExternalInput")
    sel2T_d = nc.dram_tensor("sel2T", [128, SUB2], dt.int32, kind="ExternalInput")
    cdefs = dict(wu0p=[128, 64], wu1p=[128, 64],
                 repl64=[64, 128], maski=[64, 512], mask=[128, 512], bu0p=[64, 1], bu1p=[64, 1],
                 bc0p=[128, 1], wf=[128, 40], bf=[40, 1], ident=[128, 128],
                 identb=[128, 128])
    for l in range(6):
        cdefs[f"wcp{l}"] = [128, 1024]
        cdefs[f"wcq{l}"] = [128, 1024]
    R_KEYS = {"ident", "wf"} | {
        f"wcp{l}" for l in range(6)} | {f"wcq{l}" for l in range(6)}
    B_KEYS = {"wu0p", "wu1p", "repl64", "maski", "mask", "identb"}
    cdt = lambda k: (dt.float32r if k in R_KEYS
                     else dt.bfloat16 if k in B_KEYS else dt.float32)
    cdram = {k: nc.dram_tensor(k, sh, cdt(k), kind="ExternalInput")
             for k, sh in cdefs.items()}
    ne_d = nc.dram_tensor("ne_d", [B_C, 128], dt.bfloat16)
    out_d = nc.dram_tensor("out", [NC_N, NCLS], dt.float32, kind="ExternalOutput")

    with tile.TileContext(nc) as tc, \
         nc.allow_low_precision(reason="bf16 pipeline fits the 2e-2 gate"):
        with tc.tile_pool(name="csb", bufs=1) as csb, \
             tc.tile_pool(name="work", bufs=5) as work, \
             tc.tile_pool(name="psum", bufs=1, space="PSUM") as psum:
            cst = {}
            for k, sh in cdefs.items():
                cst[k] = csb.tile(sh, cdt(k), tag=k, name=k)
                nc.sync.dma_start(out=cst[k][:], in_=cdram[k][:])
            selT_sb = csb.tile([128, SUB1], dt.int32, tag="selT")
            nc.sync.dma_start(out=selT_sb[:], in_=selT_d[:])
            sel2T_sb = csb.tile([128, SUB2], dt.int32, tag="sel2T")
            nc.sync.dma_start(out=sel2T_sb[:], in_=sel2T_d[:])
            pools = (csb, psum, work)

            _emit_layer(nc, tc, pools, cst, fp_d, selT_sb, SUB1, cst["wu0p"],
                        cst["bu0p"], [cst[f"wcp{l}"] for l in range(6)], True,
                        cst["bc0p"], ne_d, mybir, bass)
            embs = _emit_layer(nc, tc, pools, cst, ne_d, sel2T_sb, SUB2,
                               cst["wu1p"], cst["bu1p"],
                               [cst[f"wcq{l}"] for l in range(6)],
                               False, None, None, mybir, bass)

            log_ps = psum.tile([40, 512], dt.float32, tag="c_ps", name="log_ps")
            nc.tensor.matmul(out=log_ps[:], lhsT=cst["wf"][:], rhs=embs[:],
                             start=True, stop=True)
            l_sb = work.tile([40, 512], dt.float32r, tag="l_sb", name="l_sb")
            nc.vector.tensor_add(out=l_sb[:], in0=log_ps[:],
                                 in1=cst["bf"][:].to_broadcast([40, 512]))
            for c4 in range(4):
                lt_ps = psum.tile([128, 40], dt.float32r, tag="nt_ps", name="lt_ps")
                nc.tensor.transpose(out=lt_ps[:], in_=l_sb[:, c4 * 128:(c4 + 1) * 128],
                                    identity=cst["ident"][:40, :40])
                lt = work.tile([128, 40], dt.float32, tag="lt", name="lt")
                nc.scalar.copy(out=lt[:], in_=lt_ps[:])
                mx2 = work.tile([128, 1], dt.float32, tag="mx2", name="mx2")
                nc.vector.tensor_reduce(out=mx2[:], in_=lt[:],
                                        axis=mybir.AxisListType.X,
                                        op=mybir.AluOpType.max)
                nmx = work.tile([128, 1], dt.float32, tag="nmx", name="nmx")
                nc.vector.tensor_scalar_mul(out=nmx[:], in0=mx2[:], scalar1=-1.0)
                ex = work.tile([128, 40], dt.float32, tag="ex", name="ex")
                nc.scalar.activation(out=ex[:], in_=lt[:],
                                     func=mybir.ActivationFunctionType.Exp,
                                     bias=nmx[:], scale=1.0)
                es = work.tile([128, 1], dt.float32, tag="es", name="es")
                nc.vector.tensor_reduce(out=es[:], in_=ex[:],
                                        axis=mybir.AxisListType.X,
                                        op=mybir.AluOpType.add)
                lg2 = work.tile([128, 1], dt.float32, tag="lg2", name="lg2")
                nc.scalar.activation(out=lg2[:], in_=es[:],
                                     func=mybir.ActivationFunctionType.Ln)
                o1 = work.tile([128, 40], dt.float32, tag="o1", name="o1")
                nc.vector.tensor_sub(out=o1[:], in0=lt[:],
                                     in1=mx2[:].to_broadcast([128, 40]))
                nc.vector.tensor_sub(out=o1[:], in0=o1[:],
                                     in1=lg2[:].to_broadcast([128, 40]))
                nc.sync.dma_start(out=out_d[c4 * 128:(c4 + 1) * 128, :], in_=o1[:])
    nc.compile()
    return nc


def kernel(**inputs):
    from concourse.bass_utils import run_bass_kernel_spmd
    cst = _consts(inputs)
    per_core = _indices(inputs)
    if "nc" not in _CACHE:
        _CACHE["nc"] = _build()
    nc = _CACHE["nc"]
    names = ["wu0p", "wu1p", "repl64",
             "maski", "mask", "bu0p", "bu1p", "bc0p", "wf", "bf", "ident",
             "identb"]
    names += [f"wcp{l}" for l in range(6)] + [f"wcq{l}" for l in range(6)]
    in_maps = []
    for c in range(NCORES):
        m = {"fp": cst["fp"], "selT": per_core[c][0], "sel2T": per_core[c][1]}
        for k in names:
            m[k] = cst[k]
        in_maps.append(m)
    res = run_bass_kernel_spmd(nc, in_maps, list(range(NCORES)))
    return np.concatenate([res.results[c]["out"] for c in range(NCORES)], axis=0)


if __name__ == "__main__":
    pass


def kernel_traced(**inputs):
    """Rerun with NTFF tracing; returns max per-core exec ns."""
    from concourse.bass_utils import run_bass_kernel_spmd
    cst = _consts(inputs)
    per_core = _indices(inputs)
    if "nc" not in _CACHE:
        _CACHE["nc"] = _build()
    nc = _CACHE["nc"]
    names = ["wu0p", "wu1p", "repl64",
             "maski", "mask", "bu0p", "bu1p", "bc0p", "wf", "bf", "ident",
             "identb"]
    names += [f"wcp{l}" for l in range(6)] + [f"wcq{l}" for l in range(6)]
    in_maps = []
    for c in range(NCORES):
        m = {"fp": cst["fp"], "selT": per_core[c][0], "sel2T": per_core[c][1]}
        for k in names:
            m[k] = cst[k]
        in_maps.append(m)
    res = run_bass_kernel_spmd(nc, in_maps, list(range(NCORES)), trace=True)
    return res.exec_time_ns

